# revision 1
# baseline (speedup 1.0000x reference)
"""Binarized 3x3 conv (BinarizeConv2dSDP) for one TRN2 chip (8 NeuronCores).

Reference computation:
    out = conv2d(sign(x), sign(M), stride=1, pad=1) * Alpha      (all fp32)
    x: (32, 256, 56, 56)   M: (256, 256, 3, 3)   Alpha: (256, 1, 1)

Strategy (per the data-parallel sharding hint):
  - Shard x over batch: 4 images per core; replicate M/Alpha on every core.
  - On-core: binarize x and M to fp8 (+/-1 exactly representable), run the
    conv as 9 shifted DoubleRow matmuls (contraction = 256 channels in one
    pass: 128 partitions x 2 pair-rows) accumulating in PSUM, scale by
    Alpha while evacuating PSUM, DMA out fp32.
  - Activations live in SBUF as zero-padded 58x58 images so every (kh,kw)
    tap of the 3x3 kernel is just a flat column offset; one matmul computes
    an 8-output-row strip (8*58 = 464 psum columns, garbage columns at the
    row seams are simply not copied out).
"""

import time

import numpy as np

import concourse.bacc as bacc
import concourse.bass as bass
import concourse.tile as tile
from concourse import masks, mybir
from concourse.bass_utils import run_bass_kernel_spmd

F32 = mybir.dt.float32
BF16 = mybir.dt.bfloat16
FP8 = mybir.dt.float8e4

# ---- problem geometry (hardcoded; kernel.py must be self-contained) ----
N_CORES = 8
NB = 4          # images per core (32 / 8)
C = 256         # in channels  (2 halves of 128 partitions)
O = 256         # out channels (2 tiles of 128 partitions)
H = W = 56
K = 3
PW = H + 2      # padded row width  (58)
NPIX = PW * PW  # padded image size (3364)
PH = 3376       # padded image allocation (multiple of 16 for DoubleRow step)
RS = 8          # output rows per strip
NSTRIP = H // RS        # 7
NCOL = RS * PW          # 464 psum columns per strip (<= 512 fp32 bank)
NVAL = RS * W           # 448 valid columns per strip


def build_nc(paired: bool = True) -> bass.Bass:
    """Build the SPMD Bass program for one core's shard.

    paired=True : fp8 DoubleRow matmuls (K=256 per instruction, 9 per strip)
    paired=False: plain matmuls (K=128, 18 per strip) - debug fallback

    Schedule shape (engine streams follow trace order):
      sync : alpha, 4x w-DMA, 8x x-DMA, out-DMAs
      PE   : 72 warmup MMs, 18 transposes (ot=0), conv pass ot=0
             (18 transposes for ot=1 slipped in warm, mid-pass), conv ot=1
      ACT  : w-signs (ot=0), image 0/1 signs, w-signs (ot=1)
      DVE  : wz+act memsets, wbuf casts, evacuations, image 2/3 converts
             (deferred between early evacuations)
    """
    act_dt = FP8 if paired else BF16
    nc = bacc.Bacc("TRN2")

    x = nc.declare_dram_parameter("x", [NB, C, H, W], F32, isOutput=False)
    m = nc.declare_dram_parameter("m", [O, C, K, K], F32, isOutput=False)
    alpha = nc.declare_dram_parameter("alpha", [O], F32, isOutput=False)
    out = nc.declare_dram_parameter("out", [NB, O, H, W], F32, isOutput=True)

    with tile.TileContext(nc) as tc:
        with (
            tc.tile_pool(name="consts", bufs=1) as consts,
            tc.tile_pool(name="xsrc", bufs=6) as xsrc_pool,
            tc.tile_pool(name="xsrc0", bufs=4) as xsrc0_pool,
            tc.tile_pool(name="wsrc", bufs=4) as wsrc_pool,
            tc.tile_pool(name="wsgn", bufs=4) as wsgn_pool,
            tc.tile_pool(name="osb", bufs=6) as osb_pool,
            tc.tile_pool(name="ptr", bufs=2, space="PSUM") as ptr_pool,
            tc.tile_pool(name="pmm", bufs=6, space="PSUM") as pmm_pool,
        ):
            # alpha: two tiny scattered DMAs; trigger them first so they get
            # queue credit before the big transfers monopolize the DGE ring
            alpha_sb = consts.tile([128, 2], F32)
            for ot in range(2):
                nc.gpsimd.dma_start(
                    out=alpha_sb[:, ot : ot + 1],
                    in_=alpha.rearrange("(t o) -> t o", t=2)[ot].rearrange(
                        "(o u) -> o u", u=1
                    ),
                )

            # ---- weight DMAs: one per (ot, half) quarter of M ----
            # wbuf[c2, half, k*256 + ot*128 + o] = sign(M[ot*128+o, half*128+c2, kh, kw])
            wbuf = consts.tile([128, 2, K * K * O], act_dt)

            def w_dma(ot, half):
                wsrc = wsrc_pool.tile([128, 128 * K * K], F32)
                nc.sync.dma_start(
                    out=wsrc[:],
                    in_=m[
                        ot * 128 : (ot + 1) * 128, half * 128 : (half + 1) * 128
                    ].rearrange("o c kh kw -> o (c kh kw)"),
                )
                return (ot, half, wsrc)

            def x_dma(n, half):
                xs = xsrc_pool.tile([128, H * W], F32)
                nc.sync.dma_start(
                    out=xs[:],
                    in_=x[n, half * 128 : (half + 1) * 128].rearrange(
                        "c h w -> c (h w)"
                    ),
                )
                return (n, half, xs)

            def x_dma_chunk(n, half, r0, nr):
                xs = xsrc0_pool.tile([128, (H // 2) * W], F32)
                nc.sync.dma_start(
                    out=xs[: , : nr * W],
                    in_=x[n, half * 128 : (half + 1) * 128, r0 : r0 + nr, :].rearrange(
                        "c h w -> c (h w)"
                    ),
                )
                return (n, half, r0, nr, xs)

            # DGE drains transfers roughly in issue order at full aggregate
            # bandwidth, so issue order = criticality: weights for the ot=0
            # transposes, then image 0 (its top rows first - strip s only
            # needs rows 8s-1..8s+9), then the rest. Each x load has its own
            # buffer (no slot reuse -> no DMA-waits-on-sign ladder).
            HT = H // 2
            wprep = [w_dma(0, 0), w_dma(0, 1)]
            xchunks = [x_dma_chunk(0, 0, 0, HT), x_dma_chunk(0, 1, 0, HT)]
            xchunks += [x_dma_chunk(0, 0, HT, HT), x_dma_chunk(0, 1, HT, HT)]
            wprep += [w_dma(1, 0), w_dma(1, 1)]
            xtiles = []
            for n in range(1, NB):
                for half in range(2):
                    xtiles.append(x_dma(n, half))

            # ---- PE warm-up: dependency-free matmuls so the HAM clock gate
            # reaches 2.4 GHz before the real matmuls start ----
            wz = consts.tile([128, 256], BF16)
            nc.vector.memset(wz[:], 0)
            pwarm = pmm_pool.tile([128, NCOL], F32, tag="pm")
            for _ in range(40):
                nc.tensor.matmul(
                    pwarm[:, :256], wz[:, :128], wz[:], start=True, stop=True
                )

            # activation buffer: zero borders (DVE, during the prologue)
            act = consts.tile([128, 2 * NB, PH], act_dt)
            for n in range(NB):
                nc.vector.memset(
                    act[:, 2 * n : 2 * n + 2, :]
                    .rearrange("p a b -> p (a b)")
                    .bitcast(mybir.dt.uint32),
                    0,
                )

            # early tile: image-0 rows 0..26 only. Strips 0-2 of image 0
            # read from here, so their dependency interval covers just the
            # top-chunk signs (1.6MB of input) instead of the whole image -
            # conv can start ~5us earlier.
            EROWS = 27
            EPH = 1568  # 27*58 rounded up to a multiple of 16
            acte = consts.tile([128, 2, EPH], act_dt)
            nc.vector.memset(
                acte.rearrange("p a b -> p (a b)").bitcast(mybir.dt.uint32), 0
            )

            identity = consts.tile([128, 128], BF16)
            masks.make_identity(nc, identity[:])

            def w_unit(ot, half, wsrc):
                """sign + 9 PE transposes + 9 DVE casts for one M quarter."""
                wsgn = wsgn_pool.tile([128, 128 * K * K], BF16)
                nc.scalar.sign(wsgn[:], wsrc[:])
                wsgn_ck = wsgn.rearrange("o (c k) -> o c k", k=K * K)
                for kk in range(K * K):
                    tp = ptr_pool.tile([128, 128], BF16)
                    nc.tensor.transpose(tp[:], wsgn_ck[:, :, kk], identity[:])
                    nc.vector.tensor_copy(
                        wbuf[:, half, kk * O + ot * 128 : kk * O + ot * 128 + 128],
                        tp[:],
                    )

            def sign_chunk(n, half, r0, nr, xs):
                dst = act[:, 2 * n + half, : PW * PW].rearrange(
                    "p (h w) -> p h w", w=PW
                )[:, 1 + r0 : 1 + r0 + nr, 1 : W + 1]
                nc.scalar.sign(
                    dst, xs[:, : nr * W].rearrange("p (h w) -> p h w", w=W)
                )

            # ot=0 weight tiles now: these 18 transposes gate the first conv
            w_unit(*wprep[0])
            w_unit(*wprep[1])

            # image-0 top chunks -> early tile rows 1..26 (input rows 0..25)
            for half in range(2):
                _, _, _, _, xs = xchunks[half]
                dst = acte[:, half, : EROWS * PW].rearrange(
                    "p (h w) -> p h w", w=PW
                )[:, 1 : EROWS, 1 : W + 1]
                nc.scalar.sign(
                    dst,
                    xs[:, : (EROWS - 1) * W].rearrange("p (h w) -> p h w", w=W),
                )

            # main act tile, image 0: rows 23..27 from the top chunks (strips
            # s>=3 read main rows >=24), then the bottom chunks
            for half in range(2):
                _, _, _, _, xs = xchunks[half]
                dst = act[:, half, : PW * PW].rearrange("p (h w) -> p h w", w=PW)[
                    :, 24 : 29, 1 : W + 1
                ]
                nc.scalar.sign(
                    dst,
                    xs[:, 23 * W : 28 * W].rearrange("p (h w) -> p h w", w=W),
                )
            for ch in xchunks[2:]:
                sign_chunk(*ch)

            # second warm-up burst: fills the PE-idle window while image 0's
            # DMA+sign completes, so conv starts at full clock
            pwarm2 = pmm_pool.tile([128, NCOL], F32, tag="pm")
            for _ in range(46):
                nc.tensor.matmul(
                    pwarm2[:, :256], wz[:, :128], wz[:], start=True, stop=True
                )

            # ---- activations: zero-padded, binarized ----
            # act[c2, 2*n + half, ph*58 + pw] = sign(x[n, half*128+c2, ph-1, pw-1])
            for n, half, xs in xtiles:
                dst = act[:, 2 * n + half, : PW * PW].rearrange(
                    "p (h w) -> p h w", w=PW
                )[:, 1 : H + 1, 1 : W + 1]
                nc.scalar.sign(dst, xs.rearrange("p (h w) -> p h w", w=W))

            # ot=1 weight tiles: emitted mid-pass (after image 0's groups) so
            # the PE runs them warm, off the startup critical path
            wunits = [lambda u=u: w_unit(*u) for u in wprep[2:]]

            # ---- main conv loop. Image-major with ot outer within each
            # image: the first conv group needs only 18 transposes + image 0,
            # and image n is not needed until ~20 + 24.7*n us ----
            gidx = 0
            for n in range(NB):
                for ot in range(2):
                    for s in range(NSTRIP):
                        pm = pmm_pool.tile([128, NCOL], F32)
                        mm = 0
                        for kk in range(K * K):
                            kh, kw = divmod(kk, K)
                            base = (RS * s + kh) * PW + kw
                            lhsT2 = wbuf[:, :, kk * O + ot * 128 : kk * O + ot * 128 + 128]
                            NCW = NCOL - 2  # col 461 is the last valid output
                            if n == 0 and s < 3:
                                rhs2 = acte[:, :, base : base + NCW]
                            else:
                                rhs2 = act[:, 2 * n : 2 * n + 2, base : base + NCW]
                            if paired:
                                nc.tensor.matmul(
                                    pm[:, :NCW],
                                    lhsT2,
                                    rhs2,
                                    start=(mm == 0),
                                    stop=(kk == K * K - 1),
                                    perf_mode=mybir.MatmulPerfMode.DoubleRow,
                                )
                                mm += 1
                            else:
                                for half in range(2):
                                    nc.tensor.matmul(
                                        pm[:, :NCW],
                                        lhsT2[:, half, :],
                                        rhs2[:, half, :],
                                        start=(mm == 0),
                                        stop=(kk == K * K - 1 and half == 1),
                                    )
                                    mm += 1
                        # evacuate valid columns, scaled by per-channel alpha
                        # (2x extra for images binarized to +/-0.5 on DVE)
                        osb = osb_pool.tile([128, NVAL], F32)
                        nc.vector.tensor_scalar_mul(
                            osb.rearrange("p (r w) -> p r w", w=W),
                            pm.rearrange("p (r w) -> p r w", w=PW)[:, :, :W],
                            alpha_sb[:, ot : ot + 1],
                        )
                        # Early out-DMAs go through GpSimd's DGE ring: the
                        # sync HWDGE ring is saturated by the big input
                        # transfers at first, and waiting for ring credit
                        # there stalls the osb->evac->PSUM->PE chain. Late
                        # out-DMAs return to the (faster) HWDGE ring, which
                        # is idle once the inputs are in - the SWDGE ring is
                        # slow to drain the final transfers.
                        eng = nc.gpsimd if gidx < 24 else nc.sync
                        eng.dma_start(
                            out=out[
                                n, ot * 128 : (ot + 1) * 128, RS * s : RS * (s + 1), :
                            ].rearrange("o h w -> o (h w)"),
                            in_=osb[:],
                        )
                        if gidx in (3, 5) and wunits:
                            wunits.pop(0)()
                        gidx += 1
    nc.finalize()
    return nc


_NC_CACHE: dict[bool, bass.Bass] = {}


def get_nc(paired: bool = True) -> bass.Bass:
    if paired not in _NC_CACHE:
        _NC_CACHE[paired] = build_nc(paired)
    return _NC_CACHE[paired]


def kernel(x: np.ndarray, M: np.ndarray, Alpha: np.ndarray) -> np.ndarray:
    """Full (unsharded) inputs in, full output out. Runs on 8 NeuronCores."""
    assert x.shape == (N_CORES * NB, C, H, W), x.shape
    nc = get_nc(paired=True)
    x = np.ascontiguousarray(x, dtype=np.float32)
    M = np.ascontiguousarray(M, dtype=np.float32)
    a = np.ascontiguousarray(Alpha, dtype=np.float32).reshape(O)
    in_maps = [
        {"x": x[i * NB : (i + 1) * NB], "m": M, "alpha": a} for i in range(N_CORES)
    ]
    last_err = None
    for attempt in range(3):
        try:
            res = run_bass_kernel_spmd(nc, in_maps, list(range(N_CORES)))
            break
        except Exception as e:  # transient NRT/axon faults recover on retry
            last_err = e
            time.sleep(10 * (attempt + 1))
    else:
        raise last_err
    return np.concatenate([res.results[i]["out"] for i in range(N_CORES)], axis=0)



# revision 2
# speedup vs baseline: 1.1650x; 1.1650x over previous
"""Binarized 3x3 conv (BinarizeConv2dSDP) for one TRN2 chip (8 NeuronCores).

Reference computation:
    out = conv2d(sign(x), sign(M), stride=1, pad=1) * Alpha      (all fp32)
    x: (32, 256, 56, 56)   M: (256, 256, 3, 3)   Alpha: (256, 1, 1)

Strategy (per the data-parallel sharding hint):
  - Shard x over batch: 4 images per core; replicate M/Alpha on every core.
  - On-core: binarize x and M to fp8 (+/-1 exactly representable), run the
    conv as 9 shifted DoubleRow matmuls (contraction = 256 channels in one
    pass: 128 partitions x 2 pair-rows) accumulating in PSUM, scale by
    Alpha while evacuating PSUM, DMA out fp32.
  - Activations live in SBUF as zero-padded 58x58 images. Each matmul's
    moving AP is 2-level [8 rows x 56 cols] (row stride 58), so only the
    448 valid output columns of an 8-row strip are computed - no seam
    garbage (the ISA ifmap AP natively has a third dim for this).
  - Startup is latency-tuned: weights (ot=0) land first on the sync HWDGE
    queue, image-0 arrives as 9/26/23-row chunks (first chunks on the
    scalar HWDGE queue so both queues ramp in parallel), act-tile zero
    fills run on GpSimd off the DVE cast path, and a short warmup burst
    ramps the HAM clock gate so the first conv strip runs at 2.4 GHz.
"""

import time

import numpy as np

import concourse.bacc as bacc
import concourse.bass as bass
import concourse.tile as tile
from concourse import masks, mybir
from concourse.bass_utils import run_bass_kernel_spmd

F32 = mybir.dt.float32
BF16 = mybir.dt.bfloat16
FP8 = mybir.dt.float8e4

# ---- problem geometry (hardcoded; kernel.py must be self-contained) ----
N_CORES = 8
NB = 4          # images per core (32 / 8)
C = 256         # in channels  (2 halves of 128 partitions)
O = 256         # out channels (2 tiles of 128 partitions)
H = W = 56
K = 3
PW = H + 2      # padded row width  (58)
NPIX = PW * PW  # padded image size (3364)
PH = 3376       # padded image allocation (multiple of 16)
RS = 8          # output rows per strip
NSTRIP = H // RS        # 7
NVAL = RS * W           # 448 psum columns per strip (valid only)

# image-0 chunking (input-row ranges) for early conv start
U_R0, U_NR = 0, 9       # ultra chunk -> early tile, strip 0
B_R0, B_NR = 7, 26      # covers padded rows 8..33 (strips 1-3, + s4 top)
C_R0, C_NR = 33, 23     # covers padded rows 34..56 (strips 4-6)
EROWS = U_NR + 1        # early tile padded rows 0..9
EPH = 592               # 10*58=580 rounded up to a multiple of 16

N_WARM = 14             # warmup matmuls (HAM ramp; PE idle 7.7-13us anyway)
OUT_SPLIT = 12          # out-DMAs before this gidx go via GpSimd SWDGE


def build_nc() -> bass.Bass:
    """Build the SPMD Bass program for one core's shard."""
    nc = bacc.Bacc("TRN2")

    x = nc.declare_dram_parameter("x", [NB, C, H, W], F32, isOutput=False)
    m = nc.declare_dram_parameter("m", [O, C, K, K], F32, isOutput=False)
    alpha = nc.declare_dram_parameter("alpha", [O], F32, isOutput=False)
    out = nc.declare_dram_parameter("out", [NB, O, H, W], F32, isOutput=True)

    with tile.TileContext(nc) as tc:
        with (
            tc.tile_pool(name="consts", bufs=1) as consts,
            tc.tile_pool(name="xsrc", bufs=6) as xsrc_pool,
            tc.tile_pool(name="xsrc0", bufs=6) as xsrc0_pool,
            tc.tile_pool(name="wsrc", bufs=2) as wsrc_pool,
            tc.tile_pool(name="wsgn", bufs=4) as wsgn_pool,
            tc.tile_pool(name="osb", bufs=8) as osb_pool,
            tc.tile_pool(name="ptr", bufs=2, space="PSUM") as ptr_pool,
            tc.tile_pool(name="pmm", bufs=6, space="PSUM") as pmm_pool,
        ):
            # alpha: two tiny scattered DMAs on the GpSimd SWDGE ring
            alpha_sb = consts.tile([128, 2], F32)
            for ot in range(2):
                nc.gpsimd.dma_start(
                    out=alpha_sb[:, ot : ot + 1],
                    in_=alpha.rearrange("(t o) -> t o", t=2)[ot].rearrange(
                        "(o u) -> o u", u=1
                    ),
                )

            # wz before any DVE dma dispatch: warmup matmuls gate on it
            wz = consts.tile([128, 256], BF16)
            nc.vector.memset(wz[:], 0)

            # ---- weight DMAs: one contiguous 1.18MB DMA per ot block ----
            # wsrc[o, c*9 + kk] = M[ot*128+o, c, kh, kw]
            def w_dma(ot):
                wsrc = wsrc_pool.tile([128, C * K * K], F32)
                nc.sync.dma_start(
                    out=wsrc[:],
                    in_=m[ot * 128 : (ot + 1) * 128].rearrange(
                        "o c kh kw -> o (c kh kw)"
                    ),
                )
                return (ot, wsrc)

            def x_chunk_dma(eng, n, half, r0, nr):
                xs = xsrc0_pool.tile([128, B_NR * W], F32)
                eng.dma_start(
                    out=xs[:, : nr * W],
                    in_=x[n, half * 128 : (half + 1) * 128, r0 : r0 + nr, :].rearrange(
                        "c h w -> c (h w)"
                    ),
                )
                return xs

            def x_img_dma(eng, n, half):
                xs = xsrc_pool.tile([128, H * W], F32)
                eng.dma_start(
                    out=xs[:],
                    in_=x[n, half * 128 : (half + 1) * 128].rearrange(
                        "c h w -> c (h w)"
                    ),
                )
                return (n, half, xs)

            # sync queue: w(ot=0) first (longest dep chain), then w(ot=1),
            # image-0 bottom chunk, images 2-3.
            wprep = [w_dma(0)]
            # scalar queue ramps in parallel: image-0 top chunks + image 1
            xu = [x_chunk_dma(nc.scalar, 0, h2, U_R0, U_NR) for h2 in range(2)]
            xb = [x_chunk_dma(nc.scalar, 0, h2, B_R0, B_NR) for h2 in range(2)]
            wprep.append(w_dma(1))
            xc = [x_chunk_dma(nc.sync, 0, h2, C_R0, C_NR) for h2 in range(2)]
            xtiles = [x_img_dma(nc.scalar, 1, h2) for h2 in range(2)]
            for n in range(2, NB):
                for h2 in range(2):
                    xtiles.append(x_img_dma(nc.sync, n, h2))

            # ---- PE warm-up: ramps the HAM clock gate (K=4/8 -> 8/8)
            # while the weight DMA + sign land ----
            pwarm = pmm_pool.tile([128, NVAL], F32, tag="pm")
            for _ in range(N_WARM):
                nc.tensor.matmul(
                    pwarm[:, :256], wz[:, :128], wz[:], start=True, stop=True
                )

            identity = consts.tile([128, 128], BF16)
            masks.make_identity(nc, identity[:])

            # activation tiles: zero fill on GpSimd (keeps DVE free for the
            # weight-cast stream). Image 0 + early tile + image 1 now;
            # images 2-3 deferred into the conv loop.
            act = consts.tile([128, 2 * NB, PH], FP8)
            acte = consts.tile([128, 2, EPH], FP8)
            nc.gpsimd.memset(
                acte.rearrange("p a b -> p (a b)").bitcast(mybir.dt.uint32), 0
            )

            def act_memset(n):
                nc.gpsimd.memset(
                    act[:, 2 * n : 2 * n + 2, :]
                    .rearrange("p a b -> p (a b)")
                    .bitcast(mybir.dt.uint32),
                    0,
                )

            act_memset(0)
            act_memset(1)

            # ---- weight prep: sign (ACT), 9 PE transposes, 9 DVE casts ----
            # wbuf[c2, half, kk*256 + ot*128 + o] = sign(M[ot*128+o, half*128+c2, kh, kw])
            wbuf = consts.tile([128, 2, K * K * O], FP8)

            def w_unit(ot, wsrc, half):
                wsgn = wsgn_pool.tile([128, 128 * K * K], BF16)
                nc.scalar.sign(wsgn[:], wsrc[:, half * 1152 : (half + 1) * 1152])
                wsgn_ck = wsgn.rearrange("o (c k) -> o c k", k=K * K)
                for kk in range(K * K):
                    tp = ptr_pool.tile([128, 128], BF16)
                    nc.tensor.transpose(tp[:], wsgn_ck[:, :, kk], identity[:])
                    nc.vector.tensor_copy(
                        wbuf[:, half, kk * O + ot * 128 : kk * O + ot * 128 + 128],
                        tp[:],
                    )

            w_unit(0, wprep[0][1], 0)
            w_unit(0, wprep[0][1], 1)

            # ---- image-0 signs: ultra -> early tile, B/C -> main act ----
            for h2 in range(2):
                dst = acte[:, h2, : EROWS * PW].rearrange("p (h w) -> p h w", w=PW)[
                    :, 1 : 1 + U_NR, 1 : W + 1
                ]
                nc.scalar.sign(
                    dst, xu[h2][:, : U_NR * W].rearrange("p (h w) -> p h w", w=W)
                )
            for h2 in range(2):
                dst = act[:, h2, :NPIX].rearrange("p (h w) -> p h w", w=PW)[
                    :, 1 + B_R0 : 1 + B_R0 + B_NR, 1 : W + 1
                ]
                nc.scalar.sign(
                    dst, xb[h2][:, : B_NR * W].rearrange("p (h w) -> p h w", w=W)
                )
            for h2 in range(2):
                dst = act[:, h2, :NPIX].rearrange("p (h w) -> p h w", w=PW)[
                    :, 1 + C_R0 : 1 + C_R0 + C_NR, 1 : W + 1
                ]
                nc.scalar.sign(
                    dst, xc[h2][:, : C_NR * W].rearrange("p (h w) -> p h w", w=W)
                )

            def sign_img(n, half, xs):
                dst = act[:, 2 * n + half, :NPIX].rearrange(
                    "p (h w) -> p h w", w=PW
                )[:, 1 : H + 1, 1 : W + 1]
                nc.scalar.sign(dst, xs.rearrange("p (h w) -> p h w", w=W))

            # deferred work hooks: (gidx -> thunk) slipped into the conv loop
            wunits = [
                lambda: w_unit(1, wprep[1][1], 0),
                lambda: w_unit(1, wprep[1][1], 1),
            ]
            imgsigns = [lambda t=t: sign_img(*t) for t in xtiles]
            hooks = {
                3: wunits[0],
                5: wunits[1],
                6: imgsigns[0],
                7: imgsigns[1],
                8: lambda: act_memset(2),
                10: imgsigns[2],
                11: imgsigns[3],
                12: lambda: act_memset(3),
                14: imgsigns[4],
                15: imgsigns[5],
            }

            # ---- main conv loop: image-major, ot inner ----
            gidx = 0
            for n in range(NB):
                for ot in range(2):
                    for s in range(NSTRIP):
                        pm = pmm_pool.tile([128, NVAL], F32)
                        for kk in range(K * K):
                            kh, kw = divmod(kk, K)
                            base = (RS * s + kh) * PW + kw
                            lhsT2 = wbuf[
                                :, :, kk * O + ot * 128 : kk * O + ot * 128 + 128
                            ]
                            src = (
                                acte
                                if (n == 0 and s == 0)
                                else act[:, 2 * n : 2 * n + 2]
                            )
                            # 2-level moving AP: 8 rows x 56 valid cols
                            rhs4 = src[:, :, base : base + RS * PW].rearrange(
                                "p a (r w) -> p a r w", w=PW
                            )[:, :, :, :W]
                            nc.tensor.matmul(
                                pm[:],
                                lhsT2,
                                rhs4,
                                start=(kk == 0),
                                stop=(kk == K * K - 1),
                                perf_mode=mybir.MatmulPerfMode.DoubleRow,
                            )
                        # evacuate, scaled by per-channel alpha
                        osb = osb_pool.tile([128, NVAL], F32)
                        nc.vector.tensor_scalar_mul(
                            osb[:], pm[:], alpha_sb[:, ot : ot + 1]
                        )
                        # early outs via GpSimd SWDGE (sync HWDGE is busy
                        # with inputs); late outs via the idle sync ring
                        eng = nc.gpsimd if gidx < OUT_SPLIT else nc.sync
                        eng.dma_start(
                            out=out[
                                n, ot * 128 : (ot + 1) * 128, RS * s : RS * (s + 1), :
                            ].rearrange("o h w -> o (h w)"),
                            in_=osb[:],
                        )
                        if gidx in hooks:
                            hooks[gidx]()
                        gidx += 1
    nc.finalize()
    return nc


_NC_CACHE: dict[bool, bass.Bass] = {}


def get_nc(paired: bool = True) -> bass.Bass:
    if paired not in _NC_CACHE:
        _NC_CACHE[paired] = build_nc()
    return _NC_CACHE[paired]


def kernel(x: np.ndarray, M: np.ndarray, Alpha: np.ndarray) -> np.ndarray:
    """Full (unsharded) inputs in, full output out. Runs on 8 NeuronCores."""
    assert x.shape == (N_CORES * NB, C, H, W), x.shape
    nc = get_nc()
    x = np.ascontiguousarray(x, dtype=np.float32)
    M = np.ascontiguousarray(M, dtype=np.float32)
    a = np.ascontiguousarray(Alpha, dtype=np.float32).reshape(O)
    in_maps = [
        {"x": x[i * NB : (i + 1) * NB], "m": M, "alpha": a} for i in range(N_CORES)
    ]
    last_err = None
    for attempt in range(3):
        try:
            res = run_bass_kernel_spmd(nc, in_maps, list(range(N_CORES)))
            break
        except Exception as e:  # transient NRT/axon faults recover on retry
            last_err = e
            time.sleep(10 * (attempt + 1))
    else:
        raise last_err
    return np.concatenate([res.results[i]["out"] for i in range(N_CORES)], axis=0)
